# revision 18
# baseline (speedup 1.0000x reference)
"""Trainium2 Bass kernel for nn_Attention_26336739459136.

Reference computation (all fp32):
    t = s + TG_prompt                        # [4096, 1024]
    c = concat([t, query])                   # [8192, 1024]
    q, k, v = split(c @ W_qkv.T)             # v is UNUSED
    attn = softmax((q*S) @ (k.T*S))          # S = 1024**-0.25, full 8192x8192
    x_s = (attn[:4096, 4096:] @ query) @ W_proj_s.T
    x_q = (attn[4096:, :4096] @ s) @ W_proj_q.T
    return (x_s, x_q)

Sharding: every core owns an interleaved slice of 512 s-rows + 512 q-rows
(rows [512m, 512(m+1)) of each block), so the SPMD program is identical on
all 8 cores.  The whole on-device data path is bf16 with fp32 PSUM
accumulation (inputs converted host-side): fp32/fp32r streams through the PE
at 2 cycles/column while bf16 streams at 1, so bf16 halves matmul time; the
extra rounding lands at ~9e-3 max-rel-err vs the 2e-2 budget.  Per core, in
program order:
  proj:    kT then qT projection of its 1024 rows (contraction layouts come
           pre-transposed from the host); q gets the combined 1/32 scale.
           wqkT is loaded as ONE whole-matrix DMA (bf16 column slices would
           be 256 B rows, under the 512 B DMA line-rate threshold) and the
           16 kT result tiles accumulate in SBUF for a single writeback.
  AG#1:    AllGather of the kT shard -> full kT (fires right after the k
           writeback; overlaps the q projection and value' work).
  value':  q'_m = query_m @ W_proj_s.T, s'_m = s_m @ W_proj_q.T (the output
           projection folded through the attention matmul by associativity),
           written as bf16.
  AG#2:    AllGather of [q'_m; s'_m] -> v_all (needed only by phase 2;
           phase 2's half-0 value tiles prefetch during phase 1).
  phase 1: scores for its 1024 rows vs all 8192 keys, no max subtraction
           (scores are ~N(0, 2), exp can't overflow fp32).  Numerator-side
           blocks are computed transposed ([j, i]) and exp'd to bf16 tiles;
           denominator-only blocks are computed [i, j] and folded to
           row-sum partials via the activation accumulator.
  phase 2: numerator = E @ v' accumulated over 32 key tiles in PSUM (bf16,
           fp32 accumulate), plus a ones-column matmul for the numerator-side
           part of the row sums — one long accumulation run per PSUM bank
           (bank ping-pong between consecutive matmuls costs ~150 ns each).
           Normalize by 1/rowsum (per-partition scalar) on PSUM evacuation.

Benchmarking (test.py): a single blocking dispatch pays ~85 ms of axon RPC
latency regardless of kernel content, so the timing path runs an R-times
unrolled build of this same program (build_program(unroll=R)) with many
dispatches in flight and reports total/(P*R).
"""

import numpy as np

import concourse.bass as bass
import concourse.bacc as bacc
import concourse.mybir as mybir
import concourse.tile as tile
from concourse.bass_utils import run_bass_kernel_spmd

P = 128
C = 1024          # model dim (contraction for projections)
D = 1024          # head dim (contraction for scores)
NCORES = 8
HALF = 512        # rows of each branch owned per core
ROWS = 1024       # total rows owned per core
SCALE2 = float(C) ** -0.5   # (C**-0.25)**2 applied once to q

F32 = mybir.dt.float32
BF16 = mybir.dt.bfloat16
AX = mybir.AxisListType.X
EXP = mybir.ActivationFunctionType.Exp
COPY = mybir.ActivationFunctionType.Copy

# fp32r streams at 2 cycles/column on the PE (N=512 MM = 426 ns warm);
# bf16 streams at 1 (213 ns).  The whole on-device data path runs bf16 with
# fp32 PSUM accumulation: inputs are converted host-side, scores/projection
# operands are bf16, accuracy stays ~1e-2 max-rel-err (budget 2e-2).
F32R = mybir.dt.float32r
SCORE_DT = BF16
IN_DT = BF16


def _r(ap):
    """[N*128, F] dram view -> [128, N, F] partition-major tiles."""
    return ap.rearrange("(o p) f -> p o f", p=P)


def build_program(unroll=1, level=5):
    """Build the SPMD program.  unroll>1 repeats the whole kernel body that
    many times in one NEFF (identical computation each rep) — used only by
    the benchmark harness to amortize per-dispatch overhead; kernel() always
    uses unroll=1."""
    nc = bacc.Bacc(
        "TRN2", target_bir_lowering=False, debug=False, num_devices=NCORES
    )

    # ---- I/O ----
    aT = nc.dram_tensor("aT", [C, ROWS], IN_DT, kind="ExternalInput")
    tgT = nc.dram_tensor("tgT", [C, HALF], IN_DT, kind="ExternalInput")
    wqkT = nc.dram_tensor("wqkT", [C, 2 * D], IN_DT, kind="ExternalInput")
    wpsT = nc.dram_tensor("wpsT", [D, D], IN_DT, kind="ExternalInput")
    wpqT = nc.dram_tensor("wpqT", [D, D], IN_DT, kind="ExternalInput")
    out_s = nc.dram_tensor("out_s", [HALF, D], F32, kind="ExternalOutput")
    out_q = nc.dram_tensor("out_q", [HALF, D], F32, kind="ExternalOutput")

    # ---- collective buffers ----
    # Two sets, alternated by rep parity: in the unrolled benchmark build,
    # rep k+1's AllGathers would otherwise serialize behind rep k's readers
    # of the same buffers (WAR).  unroll=1 uses set 0 only.
    nbuf = 2 if unroll > 1 else 1
    kt_in = [
        nc.dram_tensor(f"kt_in{i}", [D, ROWS], SCORE_DT, kind="Internal")
        for i in range(nbuf)
    ]
    kt_all = [
        nc.dram_tensor(
            f"kt_all{i}", [NCORES * D, ROWS], SCORE_DT, kind="Internal",
            addr_space="Shared",
        )
        for i in range(nbuf)
    ]
    v_in = [
        nc.dram_tensor(f"v_in{i}", [ROWS, D], BF16, kind="Internal")
        for i in range(nbuf)
    ]
    v_all = [
        nc.dram_tensor(
            f"v_all{i}", [NCORES * ROWS, D], BF16, kind="Internal",
            addr_space="Shared",
        )
        for i in range(nbuf)
    ]

    with tile.TileContext(nc) as tc:
        # persistent scratch shared across reps: double-buffered tags so a
        # rep's phase-1 writes don't wait on the previous rep's readers;
        # the ones-column constant is set once.
        with tc.tile_pool(name="persist", bufs=2) as persist:
            ones_sb = persist.tile([P, 1], BF16, tag="ones", bufs=1)
            nc.vector.memset(ones_sb[:], 1.0)
            for _rep in range(unroll):
                b = _rep % nbuf
                build_rep(
                    nc, tc, _rep, aT, tgT, wqkT, wpsT, wpqT,
                    out_s, out_q, kt_in[b], kt_all[b], v_in[b], v_all[b],
                    level, persist, ones_sb,
                )
    nc.compile()
    return nc


def build_rep(nc, tc, rep, aT, tgT, wqkT, wpsT, wpqT, out_s, out_q,
              kt_in, kt_all, v_in, v_all, level=5, persist=None, ones_sb=None):
        # level: bench-only truncation. 5/4=full kernel (the graded path),
        # 3=stop after phase0, 2=no value'/AG2,
        # 1=projection only (no kt writeback/AG1), 0=empty rep.
        if True:
            # qT [d, i] for own 1024 rows, bf16, lives through phase 1
            qT = persist.tile([P, D // P, ROWS], SCORE_DT, tag="qT")
            # denominator partials: [p, i-tile(8), rank(8)]
            dparts = persist.tile([P, 8 * NCORES], F32, tag="dparts")

            if level < 1:
                return
            # ================= phase 0a: q/k projection =================
            with (
                tc.tile_pool(name=f"ph0{rep}", bufs=1) as ph0,
                tc.tile_pool(name=f"psum0{rep}", bufs=6, space="PSUM") as psum0,
            ):
                aT_sb = ph0.tile([P, C // P, ROWS], IN_DT)
                nc.sync.dma_start(aT_sb[:], _r(aT[:]))

                # cT for the s-row half = aT[:, :512] + tgT (q-half is raw aT)
                cT_half = ph0.tile([P, C // P, HALF], IN_DT)
                tgT_sb = ph0.tile([P, C // P, HALF], IN_DT)
                nc.scalar.dma_start(tgT_sb[:], _r(tgT[:]))
                for ct in range(C // P):
                    nc.vector.tensor_add(
                        cT_half[:, ct], aT_sb[:, ct, 0:HALF], tgT_sb[:, ct]
                    )

                def rhs_c(ct, ic):
                    if ic == 0:
                        return cT_half[:, ct]
                    return aT_sb[:, ct, HALF:ROWS]

                # ========== q/k projection: k first so AG#1 launches early ==========
                # One whole-wqkT load instead of 16 column-slice DMAs: a
                # [C,128] bf16 slice has 256 B rows (below the 512 B DMA
                # line-rate threshold) and each dma_start pays ~2 us fixed;
                # the full [128, 8, 2048] load moves 4 KB/partition runs.
                wqk_sb = ph0.tile([P, C // P, 2 * D], IN_DT)
                nc.scalar.dma_start(wqk_sb[:], _r(wqkT[:]))
                # k-proj results accumulate in SBUF and go out as ONE DMA
                # (16 x 256 KB writes -> 1 x 2 MB write).
                ksb = ph0.tile([P, C // P, ROWS], SCORE_DT)
                for dt_i in list(range(D // P, 2 * D // P)) + list(range(D // P)):
                    # q-half (raw aT) first: PE starts before the TG add chain
                    for ic in (1, 0):
                        ps = psum0.tile([P, HALF], F32, tag="ps0")
                        for ct in range(C // P):
                            nc.tensor.matmul(
                                ps[:],
                                (wqk_sb[:, ct, dt_i * P : (dt_i + 1) * P]),
                                (rhs_c(ct, ic)),
                                start=(ct == 0),
                                stop=(ct == C // P - 1),
                            )
                        if dt_i < D // P:
                            nc.scalar.activation(
                                qT[:, dt_i, ic * HALF : (ic + 1) * HALF],
                                ps[:],
                                COPY,
                                scale=SCALE2,
                            )
                        else:
                            kd = dt_i - D // P
                            nc.scalar.copy(
                                ksb[:, kd, ic * HALF : (ic + 1) * HALF], ps[:]
                            )
                    if dt_i == 2 * D // P - 1 and level >= 2:
                        # all k tiles ready -> single writeback, then AG#1
                        # overlaps everything after
                        nc.sync.dma_start(_r(kt_in[:]), ksb[:])
                        nc.gpsimd.collective_compute(
                            "AllGather",
                            mybir.AluOpType.bypass,
                            replica_groups=[list(range(NCORES))],
                            ins=[kt_in[:].opt()],
                            outs=[kt_all[:].opt()],
                        )

                # ============= value' shards (RAW s/query slices) =============
                # q'_m = query_m @ wps.T : lhsT = queryT slice (aT cols 512:)
                # s'_m = s_m @ wpq.T     : lhsT = sT slice (aT cols 0:512)
                if level < 3:
                    return
                vall_sb = ph0.tile([P, 8, D], BF16)
                for half_i, (lo, w_dram) in enumerate([(HALF, wpsT), (0, wpqT)]):
                    for ec in range(2):
                        wt = ph0.tile([P, C // P, HALF], IN_DT, tag="wp", bufs=2)
                        nc.scalar.dma_start(
                            wt[:], _r(w_dram[:, ec * HALF : (ec + 1) * HALF])
                        )
                        for jt in range(HALF // P):
                            ps = psum0.tile([P, HALF], F32, tag="ps0")
                            for ct in range(C // P):
                                nc.tensor.matmul(
                                    ps[:],
                                    (aT_sb[:, ct, lo + jt * P : lo + (jt + 1) * P]),
                                    (wt[:, ct]),
                                    start=(ct == 0),
                                    stop=(ct == C // P - 1),
                                )
                            nc.scalar.copy(
                                vall_sb[
                                    :,
                                    half_i * 4 + jt,
                                    ec * HALF : (ec + 1) * HALF,
                                ],
                                ps[:],
                            )
                nc.sync.dma_start(_r(v_in[:]), vall_sb[:])

                # AG#2: value' (needed only by phase 2; overlaps phase 1)
                nc.gpsimd.collective_compute(
                    "AllGather",
                    mybir.AluOpType.bypass,
                    replica_groups=[list(range(NCORES))],
                    ins=[v_in[:].opt()],
                    outs=[v_all[:].opt()],
                )

            # ================= phases 1+2 =================
            if level < 4:
                return
            with tc.tile_pool(name=f"epool{rep}", bufs=1) as epool:
                # exp'd transposed numerator scores, bf16:
                #  eC: j = q-rows (8 ranks x 4 jt), i = own s-rows
                #  eD: j = s-rows,                  i = own q-rows
                eC = epool.tile([P, 32, HALF], BF16)
                eD = epool.tile([P, 32, HALF], BF16)
                phase_12(nc, tc, rep, qT, eC, eD, dparts, ones_sb, kt_all, v_all, out_s, out_q)


def phase_12(nc, tc, rep, qT, eC, eD, dparts, ones_sb, kt_all, v_all, out_s, out_q):
            # ================= phase 1: scores + exp =================
            # vvp is opened alongside phase 1 so half 0's value tiles load
            # during the score loop (v_all is ready as soon as AG#2 lands);
            # phase 2 then starts without a DMA stall.  Half 1's tiles reuse
            # the same buffers and load behind half 0's last reads.
            with tc.tile_pool(name=f"vv{rep}", bufs=1) as vvp:
              def load_v(half_i):
                vA = vvp.tile([P, 16, D], BF16, tag="vA")
                vB = vvp.tile([P, 16, D], BF16, tag="vB")
                for r in range(NCORES):
                    vhalf = vA if r < 4 else vB
                    nc.sync.dma_start(
                        vhalf[:, (r % 4) * 4 : (r % 4 + 1) * 4, :],
                        _r(
                            v_all[
                                r * ROWS + half_i * HALF : r * ROWS
                                + half_i * HALF
                                + HALF,
                                :,
                            ]
                        ),
                    )
                return vA, vB

              v0 = load_v(0)
              with (
                tc.tile_pool(name=f"kt{rep}", bufs=2) as ktp,
                tc.tile_pool(name=f"psum1{rep}", bufs=6, space="PSUM") as psum1,
              ):
                for r in range(NCORES):
                    ktile = ktp.tile([P, D // P, ROWS], SCORE_DT, tag="kt")
                    nc.sync.dma_start(
                        ktile[:], _r(kt_all[r * D : (r + 1) * D, :])
                    )
                    # paths A/B: denominator-only blocks, [i, j] layout
                    for it in range(8):
                        jlo = 0 if it < 4 else HALF  # s-rows vs own-kind keys
                        ps = psum1.tile([P, HALF], F32, tag="ps1")
                        for dd in range(D // P):
                            nc.tensor.matmul(
                                ps[:],
                                (qT[:, dd, it * P : (it + 1) * P]),
                                (ktile[:, dd, jlo : jlo + HALF]),
                                start=(dd == 0),
                                stop=(dd == D // P - 1),
                            )
                        junk = ktp.tile([P, HALF], BF16, tag="junk", bufs=2)
                        nc.scalar.activation(
                            junk[:], ps[:], EXP, accum_out=dparts[:, it * NCORES + r : it * NCORES + r + 1]
                        )
                    # paths C/D: numerator blocks, [j, i] layout -> bf16 E
                    for path_i, (jlo, ilo, e_sb) in enumerate(
                        [(HALF, 0, eC), (0, HALF, eD)]
                    ):
                        for jt in range(4):
                            ps = psum1.tile([P, HALF], F32, tag="ps1")
                            for dd in range(D // P):
                                nc.tensor.matmul(
                                    ps[:],
                                    (ktile[:, dd, jlo + jt * P : jlo + (jt + 1) * P]),
                                    (qT[:, dd, ilo : ilo + HALF]),
                                    start=(dd == 0),
                                    stop=(dd == D // P - 1),
                                )
                            nc.scalar.activation(
                                e_sb[:, r * 4 + jt], ps[:], EXP
                            )

              # =============== phase 2: numerator + normalize ===============
              with (
                tc.tile_pool(name=f"psum2{rep}", bufs=2, space="PSUM") as psum2,
              ):
                # v_all rows decompose as (rank r, half h, jt, p); pick half h
                for half_i, (e_sb, out_t) in enumerate([(eC, out_s), (eD, out_q)]):
                    # half 0's tiles were prefetched during phase 1; half 1
                    # reuses the same buffers (loads overlap half 0's last
                    # accumulation runs via the pool's WAR tracking).
                    vA, vB = v0 if half_i == 0 else load_v(1)
                    for it in range(4):
                        it_g = half_i * 4 + it  # global i-tile for dparts
                        psA = psum2.tile([P, HALF], F32, tag="psA")
                        psB = psum2.tile([P, HALF], F32, tag="psB")
                        psO = psum2.tile([P, 1], F32, tag="psO")
                        # one accumulation run per PSUM bank (A, then B, then
                        # the ones-column) — switching banks every matmul costs
                        # ~150-180 ns each on the PSUM write port; long runs
                        # keep the PE at the warm streaming rate.  The LDW for
                        # each j is re-issued per run but hides under the
                        # previous matmul via the PE reorder window.
                        for ps, rsel in ((psA, 0), (psB, 1), (psO, 2)):
                            for j in range(32):
                                lhsT = e_sb[:, j, it * P : (it + 1) * P]
                                vhalf = vA if j < 16 else vB
                                vj = j % 16
                                st = dict(start=(j == 0), stop=(j == 31))
                                if rsel == 0:
                                    nc.tensor.matmul(
                                        ps[:], lhsT, vhalf[:, vj, 0:HALF], **st
                                    )
                                elif rsel == 1:
                                    nc.tensor.matmul(
                                        ps[:], lhsT, vhalf[:, vj, HALF:D], **st
                                    )
                                else:
                                    nc.tensor.matmul(ps[:], lhsT, ones_sb[:], **st)
                        dsum = vvp.tile([P, 1], F32, tag="dsum", bufs=3)
                        nc.vector.reduce_sum(dsum[:], dparts[:, it_g * NCORES : (it_g + 1) * NCORES], axis=AX)
                        nc.vector.tensor_add(dsum[:], dsum[:], psO[:])
                        recip = vvp.tile([P, 1], F32, tag="recip", bufs=3)
                        nc.vector.reciprocal(recip[:], dsum[:])
                        otile = vvp.tile([P, D], F32, tag="otile", bufs=3)
                        nc.scalar.activation(
                            otile[:, 0:HALF], psA[:], COPY, scale=recip[:]
                        )
                        nc.scalar.activation(
                            otile[:, HALF:D], psB[:], COPY, scale=recip[:]
                        )
                        nc.sync.dma_start(
                            out_t[it * P : (it + 1) * P, :], otile[:]
                        )


_NC_CACHE = None


def kernel(query, s, TG_prompt, W_qkv, W_proj_s, W_proj_q):
    global _NC_CACHE
    import ml_dtypes

    bf16 = ml_dtypes.bfloat16
    query = np.asarray(query, dtype=np.float32)
    s = np.asarray(s, dtype=np.float32)
    TG_prompt = np.asarray(TG_prompt, dtype=np.float32)
    W_qkv = np.asarray(W_qkv, dtype=np.float32)
    W_proj_s = np.asarray(W_proj_s, dtype=np.float32)
    W_proj_q = np.asarray(W_proj_q, dtype=np.float32)

    sT = np.ascontiguousarray(s.T).astype(bf16)
    qryT = np.ascontiguousarray(query.T).astype(bf16)
    tgT_full = np.ascontiguousarray(TG_prompt.T).astype(bf16)
    wqkT = np.ascontiguousarray(W_qkv[: 2 * D].T).astype(bf16)
    wpsT = np.ascontiguousarray(W_proj_s.T).astype(bf16)
    wpqT = np.ascontiguousarray(W_proj_q.T).astype(bf16)

    if _NC_CACHE is None:
        _NC_CACHE = build_program()
    nc = _NC_CACHE

    in_maps = []
    for m in range(NCORES):
        sl = slice(m * HALF, (m + 1) * HALF)
        in_maps.append(
            {
                "aT": np.ascontiguousarray(
                    np.concatenate([sT[:, sl], qryT[:, sl]], axis=1)
                ),
                "tgT": np.ascontiguousarray(tgT_full[:, sl]),
                "wqkT": wqkT,
                "wpsT": wpsT,
                "wpqT": wpqT,
            }
        )

    res = run_bass_kernel_spmd(nc, in_maps, core_ids=list(range(NCORES)))
    outs = res.results

    x_s = np.concatenate([outs[m]["out_s"] for m in range(NCORES)], axis=0)
    x_q = np.concatenate([outs[m]["out_q"] for m in range(NCORES)], axis=0)
    return (x_s, x_q)

